# revision 33
# baseline (speedup 1.0000x reference)
"""DilateAttention Trainium2 kernel (nn_DilateAttention), v2 bf16.

Full inputs q,k,v: [8, 192, 56, 56] fp32. Output: [8, 56, 56, 192] fp32.
Sharded data-parallel over batch B=8 across 8 NeuronCores.

Per-core layout: channels-on-partitions, bf16 datapath. Head group G0
(heads 0-3) fills 128 partitions; group G1 (heads 4-5, 64 ch) is pixel-split
across partitions (image halves stacked).

Pipeline per core:
  A: prod_j = q*k_shift (DVE/Pool bf16 2x)  ->  PE selector-reduce -> S PSUM
  exp on Act -> E bf16 SBUF;  B: selD-matmul sums, reciprocal, selN-matmul
  broadcast, normalize (E' bf16)
  C: per (group, j): PE broadcast E'->AB PSUM; Act evacuates AB to bf16 SBUF
  (or DVE reads PSUM directly); DVE/Pool multiply with v; PE identity-matmul
  accumulates over j into ACC PSUM.
  Output: Act converts ACC->bf16, DMA channel-major; host transposes to
  [B,H,W,C] fp32.
"""

import sys

for _p in ("/opt/trn_rl_repo",):
    if _p not in sys.path:
        sys.path.insert(0, _p)

import numpy as np

B = 8
C = 192
H = W = 56
HD = 32
NH = 6  # heads
KK = 9  # kernel*kernel
SCALE = HD ** -0.5
HWPIX = H * W  # 3136
HALF = HWPIX // 2  # 1568
HROWS = H // 2  # 28
SHIFTS = [(di, dj) for di in (-2, 0, 2) for dj in (-2, 0, 2)]
NROWS = KK * 12  # 108 score rows, row m = j*12 + h*2 + half

# G0 padded image geometry: rows y in [-2,58), cols x in [-2,58)
PADH = PADW = 60
ROW0 = COL0 = 2
# G1 dup geometry: [128, 32, 60]; lower p<64: y in [-2,30); upper: y in [26,58)
PADH1 = 32


def _build_consts():
    """Selector constants for the [108, 1568] score layout.

    Score row m = j*12 + h*2 + half  (j in [0,9), h in [0,6), half in {0,1}).
    """
    consts = {}
    NR = 12 * KK  # 108
    # selA0w: [128, 9, 109]; window [:, j, 1:109] = half0, [:, j, 0:108] = half1
    a = np.zeros((128, KK, NR + 1), np.float32)
    for p in range(128):
        for j in range(KK):
            a[p, j, j * 12 + (p // HD) * 2 + 1] = 1.0
    consts["selA0w"] = a.reshape(128, KK * (NR + 1))
    # selA1: [128, 9, 108] for the G1 dup prod (half encoded in partition)
    a = np.zeros((128, KK, NR), np.float32)
    for p in range(128):
        hh = (4 + (p % 64) // HD) * 2 + p // 64
        for j in range(KK):
            a[p, j, j * 12 + hh] = 1.0
    consts["selA1"] = a.reshape(128, KK * NR)
    # selB0lo/hi: [108, 9, 128] lhsT for G0 attn broadcast
    for half in (0, 1):
        b = np.zeros((NR, KK, 128), np.float32)
        for j in range(KK):
            for p in range(128):
                b[j * 12 + (p // HD) * 2 + half, j, p] = 1.0
        consts[f"selB0h{half}"] = b.reshape(NR, KK * 128)
    # selB1: [108, 9, 128] attn broadcast for G1 dup (half from partition)
    b = np.zeros((NR, KK, 128), np.float32)
    for j in range(KK):
        for p in range(128):
            b[j * 12 + (4 + (p % 64) // HD) * 2 + p // 64, j, p] = 1.0
    consts["selB1"] = b.reshape(NR, KK * 128)
    # selD: [108, 12] sum over j per (head, half)
    d = np.zeros((NR, 12), np.float32)
    for m in range(NR):
        d[m, m % 12] = 1.0
    consts["selD"] = d
    # ident: [128, 128] identity for PE pass-through accumulation
    consts["ident"] = np.eye(128, dtype=np.float32)
    # selN: [12, 108] broadcast per-(head,half) value to all j rows
    n = np.zeros((12, NR), np.float32)
    for m in range(NR):
        n[m % 12, m] = 1.0
    consts["selN"] = n
    return consts


# constants kept in plain fp32 (fp32 matmul path) instead of bf16
_F32R_CONSTS = set()


def _bank_chunks(c0, c1):
    """Split [c0,c1) at 512-element PSUM bank boundaries."""
    out = []
    while c0 < c1:
        nxt = min((c0 // 512 + 1) * 512, c1)
        out.append((c0, nxt))
        c0 = nxt
    return out


# stage-C path/engine assignment per (g, j, qi):
#   returns (evac, mult_engine): evac True -> Act copies AB psum -> bf16 sbuf,
#   mult at bf16 2x; evac False -> mult engine reads AB psum at fp32 rate.
def _c_path(i):
    # i = stream position 0..53; alternate paths along the stream so no
    # single engine saturates within a stretch.
    if i % 7 == 3:
        return (True, "gpsimd")  # 8 units on Pool (evac'd; Pool can't read PSUM)
    if i % 3 == 1:
        return (False, "vector")  # ~15 direct on DVE (PSUM operand, fp32 rate)
    return (True, "vector")  # rest evac'd, bf16 2x on DVE


# stage-A mult engine per pass index (0..26)
_A_POOL = {3, 8, 12, 16, 20, 24}


def _a_eng(idx):
    return "gpsimd" if idx in _A_POOL else "vector"


def build_module():
    import concourse.bacc as bacc
    import concourse.mybir as mybir
    import concourse.tile as tile

    fp32 = mybir.dt.float32
    bf16 = mybir.dt.bfloat16
    f32r = mybir.dt.float32r
    AL = mybir.AluOpType
    AF = mybir.ActivationFunctionType

    nc = bacc.Bacc("TRN2", target_bir_lowering=False, debug=False, num_devices=B)

    q_d = nc.dram_tensor("q", [C, H, W], bf16, kind="ExternalInput")
    k_d = nc.dram_tensor("k", [C, H, W], bf16, kind="ExternalInput")
    v_d = nc.dram_tensor("v", [C, H, W], bf16, kind="ExternalInput")
    o_d = nc.dram_tensor("o", [C, HWPIX], bf16, kind="ExternalOutput")
    consts = _build_consts()
    c_d = {
        name: nc.dram_tensor(
            name, list(arr.shape), fp32 if name in _F32R_CONSTS else bf16,
            kind="ExternalInput",
        )
        for name, arr in consts.items()
    }

    with tile.TileContext(nc) as tc:
        with (
            tc.tile_pool(name="io", bufs=2) as io_pool,
            tc.tile_pool(name="work", bufs=2) as work_pool,
            tc.tile_pool(name="tree", bufs=3) as tree_pool,
            tc.tile_pool(name="small", bufs=1) as small_pool,
        ):
            def load_g1_dup(dst_name, src_d, eng, eng2=None):
                """[128, 32, 60] dup tile: lower y in [-2,30), upper y in [26,58)."""
                t = io_pool.tile([128, PADH1, PADW], bf16, tag="kv", name=dst_name)
                nc.gpsimd.memset(t[0:64, 0:ROW0, :], 0.0)
                nc.gpsimd.memset(t[64:128, 30:32, :], 0.0)
                nc.gpsimd.memset(t[:, :, 0:COL0], 0.0)
                nc.gpsimd.memset(t[:, :, COL0 + W :], 0.0)
                eng.dma_start(t[0:64, ROW0 : ROW0 + 30, COL0 : COL0 + W], src_d[128:192, 0:30, :])
                (eng2 or eng).dma_start(t[64:128, 0:30, COL0 : COL0 + W], src_d[128:192, 26:56, :])
                return t

            def load_g1_q(eng, eng2=None):
                t = io_pool.tile([128, HROWS, W], bf16, tag="q", name="q1")
                eng.dma_start(t[0:64, :, :], q_d[128:192, 0:HROWS, :])
                (eng2 or eng).dma_start(t[64:128, :, :], q_d[128:192, HROWS:H, :])
                return t

            def load_g0_pad(dst_name, src_d, eng, eng2=None):
                t = io_pool.tile([128, PADH, PADW], bf16, tag="kv", name=dst_name)
                nc.gpsimd.memset(t[:, 0:ROW0, :], 0.0)
                nc.gpsimd.memset(t[:, ROW0 + H :, :], 0.0)
                nc.gpsimd.memset(t[:, ROW0 : ROW0 + H, 0:COL0], 0.0)
                nc.gpsimd.memset(t[:, ROW0 : ROW0 + H, COL0 + W :], 0.0)
                if eng2 is None:
                    eng.dma_start(t[:, ROW0 : ROW0 + H, COL0 : COL0 + W], src_d[0:128, :, :])
                else:
                    eng.dma_start(
                        t[:, ROW0 : ROW0 + HROWS, COL0 : COL0 + W], src_d[0:128, 0:HROWS, :]
                    )
                    eng2.dma_start(
                        t[:, ROW0 + HROWS : ROW0 + H, COL0 : COL0 + W],
                        src_d[0:128, HROWS:H, :],
                    )
                return t

            sel_sb = {}

            def load_const(name, eng):
                arr = consts[name]
                dt = fp32 if name in _F32R_CONSTS else bf16
                t = small_pool.tile(list(arr.shape), dt, tag=f"c_{name}", name=f"c_{name}")
                eng.dma_start(t[:], c_d[name][:])
                sel_sb[name] = t

            with tc.high_priority():
                load_const("selA1", nc.scalar)
                k1 = load_g1_dup("k1", k_d, nc.sync, nc.scalar)
                q1 = load_g1_q(nc.scalar, nc.sync)

            E_sb0 = small_pool.tile([NROWS, HALF // 2], bf16, tag="E0")
            E_sb1 = small_pool.tile([NROWS, HALF // 2], bf16, tag="E1")
            E_half = (E_sb0, E_sb1)

            # ---- stage A: scores. S_ps[m=(j*12+h*2+half), px] = sum_d q*k_shift
            selA1 = sel_sb["selA1"].rearrange("p (j m) -> p j m", j=KK)
            QW = HALF // 2  # 784
            SROWS = HROWS // 2  # 14

            q0 = io_pool.tile([128, H, W], bf16, tag="q", name="q0")
            nc.scalar.dma_start(q0[:], q_d[0:128, :, :])
            k0 = load_g0_pad("k0", k_d, nc.sync, nc.scalar)
            for name, eng in (
                ("selA0w", nc.scalar), ("selD", nc.sync), ("selN", nc.sync),
                ("selB0h0", nc.scalar), ("selB0h1", nc.sync),
                ("selB1", nc.sync), ("ident", nc.sync),
            ):
                load_const(name, eng)

            passes = [("g1", j) for j in range(KK)]
            for j in range(KK):
                passes += [("g0", j, 0), ("g0", j, 1)]

            def pass_views(p):
                di, dj = SHIFTS[p[1]]
                if p[0] == "g1":
                    qv = q1[:]
                    kv = k1[:, ROW0 + di : ROW0 + di + HROWS, COL0 + dj : COL0 + dj + W]
                else:
                    r0 = p[2] * HROWS
                    qv = q0[:, r0 : r0 + HROWS, :]
                    kv = k0[:, ROW0 + di + r0 : ROW0 + di + r0 + HROWS,
                            COL0 + dj : COL0 + dj + W]
                return qv, kv

            selB0h = [
                sel_sb["selB0h0"].rearrange("m (j p) -> m j p", j=KK),
                sel_sb["selB0h1"].rearrange("m (j p) -> m j p", j=KK),
            ]
            selB1 = sel_sb["selB1"].rearrange("m (j p) -> m j p", j=KK)

            with tc.tile_pool(name="psS", bufs=1, space="PSUM") as psS_pool:
                S_ps = psS_pool.tile([NROWS, HALF], fp32, tag="S")
                prods = {}
                for idx, p in enumerate(passes):
                    prod = work_pool.tile(
                        [128, HROWS, W], bf16, tag="prod", bufs=27, name=f"aprod{idx}"
                    )
                    qv, kv = pass_views(p)
                    getattr(nc, _a_eng(idx)).tensor_tensor(prod[:], qv, kv, AL.mult)
                    prods[idx] = prod

                selA0w = sel_sb["selA0w"].rearrange("p (j m) -> p j m", j=KK)
                selA0h = [selA0w[:, :, 1 : NROWS + 1], selA0w[:, :, 0:NROWS]]
                for idx, p in enumerate(passes):
                    sel = selA1[:, p[1], :] if p[0] == "g1" else selA0h[p[2]][:, p[1], :]
                    pflat = prods[idx].rearrange("p a b -> p (a b)")
                    for n0, n1 in _bank_chunks(0, HALF):
                        nc.tensor.matmul(
                            S_ps[:, n0:n1], sel, pflat[:, n0:n1],
                            start=(idx == 0), stop=(idx == len(passes) - 1),
                        )

                # prefetch v while PE drains the last score matmuls
                v0 = load_g0_pad("v0", v_d, nc.sync, nc.scalar)
                v1 = load_g1_dup("v1", v_d, nc.scalar)

                # exp(scale * S) -> E bf16, evacuating PSUM
                for s in (0, 1):
                    nc.scalar.activation(
                        E_half[s][:], S_ps[:, s * (HALF // 2) : (s + 1) * (HALF // 2)],
                        AF.Exp, scale=float(SCALE),
                    )

            with tc.tile_pool(name="psC", bufs=1, space="PSUM") as ps_pool:
                def emit_B(s):
                    """Normalize E[:, s-chunk]: selD sum, reciprocal, selN
                    broadcast, in-place multiply. Chunked at bank granularity
                    so the two sub-chains pipeline across engines. PSUM via
                    the AB slot ring."""
                    Et = E_half[s]
                    D_ps = ps_pool.tile([12, QW], fp32, tag="AB", bufs=3, name=f"D{s}")
                    RB_ps = ps_pool.tile([NROWS, QW], fp32, tag="AB", bufs=3, name=f"RB{s}")
                    R_ch = small_pool.tile([12, QW], fp32, tag="R", bufs=2)
                    Rb = small_pool.tile([12, QW], bf16, tag="Rb", bufs=2)
                    for c0, c1 in _bank_chunks(0, QW):
                        nc.tensor.matmul(
                            D_ps[:, c0:c1], sel_sb["selD"][:],
                            Et[:, c0:c1], start=True, stop=True,
                        )
                        nc.vector.reciprocal_approx_fast(R_ch[:, c0:c1], D_ps[:, c0:c1])
                        nc.scalar.activation(Rb[:, c0:c1], R_ch[:, c0:c1], AF.Copy)
                        nc.tensor.matmul(
                            RB_ps[:, c0:c1], sel_sb["selN"][:], Rb[:, c0:c1],
                            start=True, stop=True,
                        )
                        nc.vector.tensor_tensor(
                            Et[:, c0:c1], Et[:, c0:c1], RB_ps[:, c0:c1], AL.mult,
                        )

                ident = sel_sb["ident"]
                acc_tiles = {}

                def emit_front(g, j, qi, i):
                    """AB matmuls + evac + mult for unit (g,j,qi); returns prod."""
                    hf = g if g < 2 else 0
                    sel = selB1 if g == 2 else selB0h[hf]
                    vt = v1 if g == 2 else v0
                    di, dj = SHIFTS[j]
                    evac, meng = _c_path(i)
                    ab_ps = ps_pool.tile([128, QW], fp32, tag="AB", bufs=3)
                    for n0, n1 in _bank_chunks(0, QW):
                        nc.tensor.matmul(
                            ab_ps[:, n0:n1], sel[:, j, :],
                            E_half[qi][:, n0:n1], start=True, stop=True,
                        )
                    r0 = ROW0 + di + (hf * HROWS if g < 2 else 0) + qi * SROWS
                    vv = vt[:, r0 : r0 + SROWS, COL0 + dj : COL0 + dj + W]
                    prod = tree_pool.tile(
                        [128, SROWS, W], bf16, tag="prod", bufs=16, name="cprod"
                    )
                    if evac:
                        ab_sb = tree_pool.tile([128, QW], bf16, tag="absb", bufs=16)
                        nc.scalar.activation(ab_sb[:], ab_ps[:], AF.Copy)
                        m_in = ab_sb.rearrange("p (a b) -> p a b", a=SROWS)
                    else:
                        m_in = ab_ps.rearrange("p (a b) -> p a b", a=SROWS)
                    getattr(nc, meng).tensor_tensor(prod[:], m_in, vv, AL.mult)
                    return prod

                def emit_back(g, j, qi, prod):
                    """ACC accumulate for unit; on j==8 evacuate + DMA out."""
                    hf = g if g < 2 else 0
                    key = (g, qi)
                    if j == 0:
                        acc_tiles[key] = [
                            ps_pool.tile(
                                [128, n1 - n0], fp32, tag="ACCb", bufs=2,
                                name=f"ACC{g}{qi}c{n0}",
                            )
                            for n0, n1 in _bank_chunks(0, QW)
                        ]
                    pf = prod.rearrange("p a b -> p (a b)")
                    for ci, (n0, n1) in enumerate(_bank_chunks(0, QW)):
                        nc.tensor.matmul(
                            acc_tiles[key][ci][:], ident[:], pf[:, n0:n1],
                            start=(j == 0), stop=(j == KK - 1),
                        )
                    if j == KK - 1:
                        t_sb = tree_pool.tile([128, QW], bf16, tag="tout", bufs=4)
                        for ci, (n0, n1) in enumerate(_bank_chunks(0, QW)):
                            nc.scalar.activation(
                                t_sb[:, n0:n1], acc_tiles[key][ci][:], AF.Copy
                            )
                        c0 = hf * HALF + qi * QW
                        if g < 2:
                            nc.sync.dma_start(o_d[0:128, c0 : c0 + QW], t_sb[:])
                        else:
                            nc.sync.dma_start(o_d[128:192, c0 : c0 + QW], t_sb[0:64, :])
                            nc.scalar.dma_start(
                                o_d[128:192, HALF + c0 : HALF + c0 + QW], t_sb[64:128, :]
                            )

                # G1 (two output DMAs) first within each stripe so the
                # final block drains through a single output DMA chain.
                units = [(g, j, qi) for qi in (0, 1) for g in (2, 0, 1) for j in range(KK)]
                SK = 14  # pipeline skew: ACC(u) emitted SK units after AB(u)
                emit_B(0)
                queue = []
                for i, u in enumerate(units):
                    if i == 2:
                        emit_B(1)
                    sk = SK if i < 40 else 5
                    while len(queue) >= sk:
                        uu, pp = queue.pop(0)
                        emit_back(*uu, pp)
                    prod = emit_front(*u, i)
                    queue.append((u, prod))
                for uu, pp in queue:
                    emit_back(*uu, pp)

    nc.compile()
    return nc, consts


_CACHE = {}


def _get_module():
    if "nc" not in _CACHE:
        _CACHE["nc"], _CACHE["consts"] = build_module()
    return _CACHE["nc"], _CACHE["consts"]


def make_in_maps(q, k, v, consts):
    import ml_dtypes

    bf = ml_dtypes.bfloat16
    cb = {}
    for name, arr in consts.items():
        cb[name] = arr.astype(np.float32) if name in _F32R_CONSTS else arr.astype(bf)
    qb = np.asarray(q).astype(bf)
    kb = np.asarray(k).astype(bf)
    vb = np.asarray(v).astype(bf)
    in_maps = []
    for b in range(B):
        m = {
            "q": np.ascontiguousarray(qb[b].reshape(C, H, W)),
            "k": np.ascontiguousarray(kb[b].reshape(C, H, W)),
            "v": np.ascontiguousarray(vb[b].reshape(C, H, W)),
        }
        m.update(cb)
        in_maps.append(m)
    return in_maps


def postprocess(raw_o):
    """[C, HWPIX] bf16 per core -> [H, W, C] fp32."""
    return (
        np.asarray(raw_o).astype(np.float32).reshape(C, H, W).transpose(1, 2, 0)
    )


def kernel(q: np.ndarray, k: np.ndarray, v: np.ndarray) -> np.ndarray:
    from concourse import bass_utils

    nc, consts = _get_module()
    in_maps = make_in_maps(np.asarray(q), np.asarray(k), np.asarray(v), consts)
    res = bass_utils.run_bass_kernel_spmd(nc, in_maps, core_ids=list(range(B)))
    out = np.stack([postprocess(r["o"]) for r in res.results])
    return out


# revision 34
# speedup vs baseline: 1.0161x; 1.0161x over previous
"""DilateAttention Trainium2 kernel (nn_DilateAttention), v2 bf16.

Full inputs q,k,v: [8, 192, 56, 56] fp32. Output: [8, 56, 56, 192] fp32.
Sharded data-parallel over batch B=8 across 8 NeuronCores.

Per-core layout: channels-on-partitions, bf16 datapath. Head group G0
(heads 0-3) fills 128 partitions; group G1 (heads 4-5, 64 ch) is pixel-split
across partitions (image halves stacked).

Pipeline per core:
  A: prod_j = q*k_shift (DVE/Pool bf16 2x)  ->  PE selector-reduce -> S PSUM
  exp on Act -> E bf16 SBUF;  B: selD-matmul sums, reciprocal, selN-matmul
  broadcast, normalize (E' bf16)
  C: per (group, j): PE broadcast E'->AB PSUM; Act evacuates AB to bf16 SBUF
  (or DVE reads PSUM directly); DVE/Pool multiply with v; PE identity-matmul
  accumulates over j into ACC PSUM.
  Output: Act converts ACC->bf16, DMA channel-major; host transposes to
  [B,H,W,C] fp32.
"""

import sys

for _p in ("/opt/trn_rl_repo",):
    if _p not in sys.path:
        sys.path.insert(0, _p)

import numpy as np

B = 8
C = 192
H = W = 56
HD = 32
NH = 6  # heads
KK = 9  # kernel*kernel
SCALE = HD ** -0.5
HWPIX = H * W  # 3136
HALF = HWPIX // 2  # 1568
HROWS = H // 2  # 28
SHIFTS = [(di, dj) for di in (-2, 0, 2) for dj in (-2, 0, 2)]
NROWS = KK * 12  # 108 score rows, row m = j*12 + h*2 + half

# G0 padded image geometry: rows y in [-2,58), cols x in [-2,58)
PADH = PADW = 60
ROW0 = COL0 = 2
# G1 dup geometry: [128, 32, 60]; lower p<64: y in [-2,30); upper: y in [26,58)
PADH1 = 32


def _build_consts():
    """Selector constants for the [108, 1568] score layout.

    Score row m = j*12 + h*2 + half  (j in [0,9), h in [0,6), half in {0,1}).
    """
    consts = {}
    NR = 12 * KK  # 108
    # selA0w: [128, 9, 109]; window [:, j, 1:109] = half0, [:, j, 0:108] = half1
    a = np.zeros((128, KK, NR + 1), np.float32)
    for p in range(128):
        for j in range(KK):
            a[p, j, j * 12 + (p // HD) * 2 + 1] = 1.0
    consts["selA0w"] = a.reshape(128, KK * (NR + 1))
    # selA1: [128, 9, 108] for the G1 dup prod (half encoded in partition)
    a = np.zeros((128, KK, NR), np.float32)
    for p in range(128):
        hh = (4 + (p % 64) // HD) * 2 + p // 64
        for j in range(KK):
            a[p, j, j * 12 + hh] = 1.0
    consts["selA1"] = a.reshape(128, KK * NR)
    # selB0lo/hi: [108, 9, 128] lhsT for G0 attn broadcast
    for half in (0, 1):
        b = np.zeros((NR, KK, 128), np.float32)
        for j in range(KK):
            for p in range(128):
                b[j * 12 + (p // HD) * 2 + half, j, p] = 1.0
        consts[f"selB0h{half}"] = b.reshape(NR, KK * 128)
    # selB1: [108, 9, 128] attn broadcast for G1 dup (half from partition)
    b = np.zeros((NR, KK, 128), np.float32)
    for j in range(KK):
        for p in range(128):
            b[j * 12 + (4 + (p % 64) // HD) * 2 + p // 64, j, p] = 1.0
    consts["selB1"] = b.reshape(NR, KK * 128)
    # selD: [108, 12] sum over j per (head, half)
    d = np.zeros((NR, 12), np.float32)
    for m in range(NR):
        d[m, m % 12] = 1.0
    consts["selD"] = d
    # ident: [128, 128] identity for PE pass-through accumulation
    consts["ident"] = np.eye(128, dtype=np.float32)
    # selN: [12, 108] broadcast per-(head,half) value to all j rows
    n = np.zeros((12, NR), np.float32)
    for m in range(NR):
        n[m % 12, m] = 1.0
    consts["selN"] = n
    return consts


# constants kept in plain fp32 (fp32 matmul path) instead of bf16
_F32R_CONSTS = set()


def _bank_chunks(c0, c1):
    """Split [c0,c1) at 512-element PSUM bank boundaries."""
    out = []
    while c0 < c1:
        nxt = min((c0 // 512 + 1) * 512, c1)
        out.append((c0, nxt))
        c0 = nxt
    return out


# stage-C path/engine assignment per (g, j, qi):
#   returns (evac, mult_engine): evac True -> Act copies AB psum -> bf16 sbuf,
#   mult at bf16 2x; evac False -> mult engine reads AB psum at fp32 rate.
def _c_path(i):
    # i = stream position 0..53; alternate paths along the stream so no
    # single engine saturates within a stretch.
    if i % 7 == 3:
        return (True, "gpsimd")  # 8 units on Pool (evac'd; Pool can't read PSUM)
    if i % 3 == 1:
        return (False, "vector")  # ~15 direct on DVE (PSUM operand, fp32 rate)
    return (True, "vector")  # rest evac'd, bf16 2x on DVE


# stage-A mult engine per pass index (0..26)
_A_POOL = {3, 8, 12, 16, 20, 24}


def _a_eng(idx):
    return "gpsimd" if idx in _A_POOL else "vector"


def build_module():
    import concourse.bacc as bacc
    import concourse.mybir as mybir
    import concourse.tile as tile

    fp32 = mybir.dt.float32
    bf16 = mybir.dt.bfloat16
    f32r = mybir.dt.float32r
    AL = mybir.AluOpType
    AF = mybir.ActivationFunctionType

    nc = bacc.Bacc("TRN2", target_bir_lowering=False, debug=False, num_devices=B)

    q_d = nc.dram_tensor("q", [C, H, W], bf16, kind="ExternalInput")
    k_d = nc.dram_tensor("k", [C, H, W], bf16, kind="ExternalInput")
    v_d = nc.dram_tensor("v", [C, H, W], bf16, kind="ExternalInput")
    o_d = nc.dram_tensor("o", [C, HWPIX], bf16, kind="ExternalOutput")
    consts = _build_consts()
    c_d = {
        name: nc.dram_tensor(
            name, list(arr.shape), fp32 if name in _F32R_CONSTS else bf16,
            kind="ExternalInput",
        )
        for name, arr in consts.items()
    }

    with tile.TileContext(nc) as tc:
        with (
            tc.tile_pool(name="io", bufs=2) as io_pool,
            tc.tile_pool(name="work", bufs=2) as work_pool,
            tc.tile_pool(name="tree", bufs=3) as tree_pool,
            tc.tile_pool(name="small", bufs=1) as small_pool,
        ):
            def load_g1_dup(dst_name, src_d, eng, eng2=None):
                """[128, 32, 60] dup tile: lower y in [-2,30), upper y in [26,58)."""
                t = io_pool.tile([128, PADH1, PADW], bf16, tag="kv", name=dst_name)
                nc.gpsimd.memset(t[0:64, 0:ROW0, :], 0.0)
                nc.gpsimd.memset(t[64:128, 30:32, :], 0.0)
                nc.gpsimd.memset(t[:, :, 0:COL0], 0.0)
                nc.gpsimd.memset(t[:, :, COL0 + W :], 0.0)
                eng.dma_start(t[0:64, ROW0 : ROW0 + 30, COL0 : COL0 + W], src_d[128:192, 0:30, :])
                (eng2 or eng).dma_start(t[64:128, 0:30, COL0 : COL0 + W], src_d[128:192, 26:56, :])
                return t

            def load_g1_q(eng, eng2=None):
                t = io_pool.tile([128, HROWS, W], bf16, tag="q", name="q1")
                eng.dma_start(t[0:64, :, :], q_d[128:192, 0:HROWS, :])
                (eng2 or eng).dma_start(t[64:128, :, :], q_d[128:192, HROWS:H, :])
                return t

            def load_g0_pad(dst_name, src_d, eng, eng2=None):
                t = io_pool.tile([128, PADH, PADW], bf16, tag="kv", name=dst_name)
                nc.gpsimd.memset(t[:, 0:ROW0, :], 0.0)
                nc.gpsimd.memset(t[:, ROW0 + H :, :], 0.0)
                nc.gpsimd.memset(t[:, ROW0 : ROW0 + H, 0:COL0], 0.0)
                nc.gpsimd.memset(t[:, ROW0 : ROW0 + H, COL0 + W :], 0.0)
                if eng2 is None:
                    eng.dma_start(t[:, ROW0 : ROW0 + H, COL0 : COL0 + W], src_d[0:128, :, :])
                else:
                    eng.dma_start(
                        t[:, ROW0 : ROW0 + HROWS, COL0 : COL0 + W], src_d[0:128, 0:HROWS, :]
                    )
                    eng2.dma_start(
                        t[:, ROW0 + HROWS : ROW0 + H, COL0 : COL0 + W],
                        src_d[0:128, HROWS:H, :],
                    )
                return t

            sel_sb = {}

            def load_const(name, eng):
                arr = consts[name]
                dt = fp32 if name in _F32R_CONSTS else bf16
                t = small_pool.tile(list(arr.shape), dt, tag=f"c_{name}", name=f"c_{name}")
                eng.dma_start(t[:], c_d[name][:])
                sel_sb[name] = t

            with tc.high_priority():
                load_const("selA1", nc.scalar)
                k1 = load_g1_dup("k1", k_d, nc.sync, nc.scalar)
                q1 = load_g1_q(nc.scalar, nc.sync)

            E_sb0 = small_pool.tile([NROWS, HALF // 2], bf16, tag="E0")
            E_sb1 = small_pool.tile([NROWS, HALF // 2], bf16, tag="E1")
            E_half = (E_sb0, E_sb1)

            # ---- stage A: scores. S_ps[m=(j*12+h*2+half), px] = sum_d q*k_shift
            selA1 = sel_sb["selA1"].rearrange("p (j m) -> p j m", j=KK)
            QW = HALF // 2  # 784
            SROWS = HROWS // 2  # 14

            q0 = io_pool.tile([128, H, W], bf16, tag="q", name="q0")
            nc.scalar.dma_start(q0[:], q_d[0:128, :, :])
            k0 = load_g0_pad("k0", k_d, nc.sync, nc.scalar)
            for name, eng in (
                ("selA0w", nc.scalar), ("selD", nc.sync), ("selN", nc.sync),
                ("selB0h0", nc.scalar), ("selB0h1", nc.sync),
                ("selB1", nc.sync), ("ident", nc.sync),
            ):
                load_const(name, eng)

            passes = [("g1", j) for j in range(KK)]
            for j in range(KK):
                passes += [("g0", j, 0), ("g0", j, 1)]

            def pass_views(p):
                di, dj = SHIFTS[p[1]]
                if p[0] == "g1":
                    qv = q1[:]
                    kv = k1[:, ROW0 + di : ROW0 + di + HROWS, COL0 + dj : COL0 + dj + W]
                else:
                    r0 = p[2] * HROWS
                    qv = q0[:, r0 : r0 + HROWS, :]
                    kv = k0[:, ROW0 + di + r0 : ROW0 + di + r0 + HROWS,
                            COL0 + dj : COL0 + dj + W]
                return qv, kv

            selB0h = [
                sel_sb["selB0h0"].rearrange("m (j p) -> m j p", j=KK),
                sel_sb["selB0h1"].rearrange("m (j p) -> m j p", j=KK),
            ]
            selB1 = sel_sb["selB1"].rearrange("m (j p) -> m j p", j=KK)

            with tc.tile_pool(name="psS", bufs=1, space="PSUM") as psS_pool:
                S_ps = psS_pool.tile([NROWS, HALF], fp32, tag="S")
                prods = {}
                for idx, p in enumerate(passes):
                    prod = work_pool.tile(
                        [128, HROWS, W], bf16, tag="prod", bufs=27, name=f"aprod{idx}"
                    )
                    qv, kv = pass_views(p)
                    getattr(nc, _a_eng(idx)).tensor_tensor(prod[:], qv, kv, AL.mult)
                    prods[idx] = prod

                selA0w = sel_sb["selA0w"].rearrange("p (j m) -> p j m", j=KK)
                selA0h = [selA0w[:, :, 1 : NROWS + 1], selA0w[:, :, 0:NROWS]]
                for idx, p in enumerate(passes):
                    sel = selA1[:, p[1], :] if p[0] == "g1" else selA0h[p[2]][:, p[1], :]
                    pflat = prods[idx].rearrange("p a b -> p (a b)")
                    for n0, n1 in _bank_chunks(0, HALF):
                        nc.tensor.matmul(
                            S_ps[:, n0:n1], sel, pflat[:, n0:n1],
                            start=(idx == 0), stop=(idx == len(passes) - 1),
                        )

                # prefetch v while PE drains the last score matmuls
                v0 = load_g0_pad("v0", v_d, nc.sync, nc.scalar)
                v1 = load_g1_dup("v1", v_d, nc.scalar)

                # exp(scale * S) -> E bf16, evacuating PSUM
                for s in (0, 1):
                    nc.scalar.activation(
                        E_half[s][:], S_ps[:, s * (HALF // 2) : (s + 1) * (HALF // 2)],
                        AF.Exp, scale=float(SCALE),
                    )

            with tc.tile_pool(name="psC", bufs=1, space="PSUM") as ps_pool:
                def emit_B(s):
                    """Normalize E[:, s-chunk]: selD sum, reciprocal, selN
                    broadcast, in-place multiply. Chunked at bank granularity
                    so the two sub-chains pipeline across engines. PSUM via
                    the AB slot ring."""
                    Et = E_half[s]
                    D_ps = ps_pool.tile([12, QW], fp32, tag="AB", bufs=3, name=f"D{s}")
                    RB_ps = ps_pool.tile([NROWS, QW], fp32, tag="AB", bufs=3, name=f"RB{s}")
                    R_ch = small_pool.tile([12, QW], fp32, tag="R", bufs=2)
                    Rb = small_pool.tile([12, QW], bf16, tag="Rb", bufs=2)
                    for c0, c1 in _bank_chunks(0, QW):
                        nc.tensor.matmul(
                            D_ps[:, c0:c1], sel_sb["selD"][:],
                            Et[:, c0:c1], start=True, stop=True,
                        )
                        nc.vector.reciprocal_approx_fast(R_ch[:, c0:c1], D_ps[:, c0:c1])
                        nc.scalar.activation(Rb[:, c0:c1], R_ch[:, c0:c1], AF.Copy)
                        nc.tensor.matmul(
                            RB_ps[:, c0:c1], sel_sb["selN"][:], Rb[:, c0:c1],
                            start=True, stop=True,
                        )
                        nc.vector.tensor_tensor(
                            Et[:, c0:c1], Et[:, c0:c1], RB_ps[:, c0:c1], AL.mult,
                        )

                ident = sel_sb["ident"]
                acc_tiles = {}

                def emit_front(g, j, qi, i):
                    """AB matmuls + evac + mult for unit (g,j,qi); returns prod."""
                    hf = g if g < 2 else 0
                    sel = selB1 if g == 2 else selB0h[hf]
                    vt = v1 if g == 2 else v0
                    di, dj = SHIFTS[j]
                    evac, meng = _c_path(i)
                    ab_ps = ps_pool.tile([128, QW], fp32, tag="AB", bufs=3)
                    for n0, n1 in _bank_chunks(0, QW):
                        nc.tensor.matmul(
                            ab_ps[:, n0:n1], sel[:, j, :],
                            E_half[qi][:, n0:n1], start=True, stop=True,
                        )
                    r0 = ROW0 + di + (hf * HROWS if g < 2 else 0) + qi * SROWS
                    vv = vt[:, r0 : r0 + SROWS, COL0 + dj : COL0 + dj + W]
                    prod = tree_pool.tile(
                        [128, SROWS, W], bf16, tag="prod", bufs=16, name="cprod"
                    )
                    if evac:
                        ab_sb = tree_pool.tile([128, QW], bf16, tag="absb", bufs=16)
                        nc.scalar.activation(ab_sb[:], ab_ps[:], AF.Copy)
                        m_in = ab_sb.rearrange("p (a b) -> p a b", a=SROWS)
                    else:
                        m_in = ab_ps.rearrange("p (a b) -> p a b", a=SROWS)
                    getattr(nc, meng).tensor_tensor(prod[:], m_in, vv, AL.mult)
                    return prod

                def emit_back(g, j, qi, prod):
                    """ACC accumulate for unit; on j==8 evacuate + DMA out."""
                    hf = g if g < 2 else 0
                    key = (g, qi)
                    if j == 0:
                        acc_tiles[key] = [
                            ps_pool.tile(
                                [128, n1 - n0], fp32, tag="ACCb", bufs=2,
                                name=f"ACC{g}{qi}c{n0}",
                            )
                            for n0, n1 in _bank_chunks(0, QW)
                        ]
                    pf = prod.rearrange("p a b -> p (a b)")
                    for ci, (n0, n1) in enumerate(_bank_chunks(0, QW)):
                        nc.tensor.matmul(
                            acc_tiles[key][ci][:], ident[:], pf[:, n0:n1],
                            start=(j == 0), stop=(j == KK - 1),
                        )
                    if j == KK - 1:
                        t_sb = tree_pool.tile([128, QW], bf16, tag="tout", bufs=4)
                        for ci, (n0, n1) in enumerate(_bank_chunks(0, QW)):
                            nc.scalar.activation(
                                t_sb[:, n0:n1], acc_tiles[key][ci][:], AF.Copy
                            )
                        c0 = hf * HALF + qi * QW
                        if g < 2:
                            nc.sync.dma_start(o_d[0:128, c0 : c0 + QW], t_sb[:])
                        else:
                            nc.sync.dma_start(o_d[128:192, c0 : c0 + QW], t_sb[0:64, :])
                            nc.scalar.dma_start(
                                o_d[128:192, HALF + c0 : HALF + c0 + QW], t_sb[64:128, :]
                            )

                # G1 (two output DMAs) first within each stripe so the
                # final block drains through a single output DMA chain.
                units = [(g, j, qi) for qi in (0, 1) for g in (2, 0, 1) for j in range(KK)]
                SK = 14  # pipeline skew: ACC(u) emitted SK units after AB(u)
                emit_B(0)
                queue = []
                for i, u in enumerate(units):
                    if i == 4:
                        emit_B(1)
                    sk = SK if i < 44 else 6
                    while len(queue) >= sk:
                        uu, pp = queue.pop(0)
                        emit_back(*uu, pp)
                    prod = emit_front(*u, i)
                    queue.append((u, prod))
                for uu, pp in queue:
                    emit_back(*uu, pp)

    nc.compile()
    return nc, consts


_CACHE = {}


def _get_module():
    if "nc" not in _CACHE:
        _CACHE["nc"], _CACHE["consts"] = build_module()
    return _CACHE["nc"], _CACHE["consts"]


def make_in_maps(q, k, v, consts):
    import ml_dtypes

    bf = ml_dtypes.bfloat16
    cb = {}
    for name, arr in consts.items():
        cb[name] = arr.astype(np.float32) if name in _F32R_CONSTS else arr.astype(bf)
    qb = np.asarray(q).astype(bf)
    kb = np.asarray(k).astype(bf)
    vb = np.asarray(v).astype(bf)
    in_maps = []
    for b in range(B):
        m = {
            "q": np.ascontiguousarray(qb[b].reshape(C, H, W)),
            "k": np.ascontiguousarray(kb[b].reshape(C, H, W)),
            "v": np.ascontiguousarray(vb[b].reshape(C, H, W)),
        }
        m.update(cb)
        in_maps.append(m)
    return in_maps


def postprocess(raw_o):
    """[C, HWPIX] bf16 per core -> [H, W, C] fp32."""
    return (
        np.asarray(raw_o).astype(np.float32).reshape(C, H, W).transpose(1, 2, 0)
    )


def kernel(q: np.ndarray, k: np.ndarray, v: np.ndarray) -> np.ndarray:
    from concourse import bass_utils

    nc, consts = _get_module()
    in_maps = make_in_maps(np.asarray(q), np.asarray(k), np.asarray(v), consts)
    res = bass_utils.run_bass_kernel_spmd(nc, in_maps, core_ids=list(range(B)))
    out = np.stack([postprocess(r["o"]) for r in res.results])
    return out


# revision 35
# speedup vs baseline: 1.0174x; 1.0013x over previous
"""DilateAttention Trainium2 kernel (nn_DilateAttention), v2 bf16.

Full inputs q,k,v: [8, 192, 56, 56] fp32. Output: [8, 56, 56, 192] fp32.
Sharded data-parallel over batch B=8 across 8 NeuronCores.

Per-core layout: channels-on-partitions, bf16 datapath. Head group G0
(heads 0-3) fills 128 partitions; group G1 (heads 4-5, 64 ch) is pixel-split
across partitions (image halves stacked).

Pipeline per core:
  A: prod_j = q*k_shift (DVE/Pool bf16 2x)  ->  PE selector-reduce -> S PSUM
  exp on Act -> E bf16 SBUF;  B: selD-matmul sums, reciprocal, selN-matmul
  broadcast, normalize (E' bf16)
  C: per (group, j): PE broadcast E'->AB PSUM; Act evacuates AB to bf16 SBUF
  (or DVE reads PSUM directly); DVE/Pool multiply with v; PE identity-matmul
  accumulates over j into ACC PSUM.
  Output: Act converts ACC->bf16, DMA channel-major; host transposes to
  [B,H,W,C] fp32.
"""

import sys

for _p in ("/opt/trn_rl_repo",):
    if _p not in sys.path:
        sys.path.insert(0, _p)

import numpy as np

B = 8
C = 192
H = W = 56
HD = 32
NH = 6  # heads
KK = 9  # kernel*kernel
SCALE = HD ** -0.5
HWPIX = H * W  # 3136
HALF = HWPIX // 2  # 1568
HROWS = H // 2  # 28
SHIFTS = [(di, dj) for di in (-2, 0, 2) for dj in (-2, 0, 2)]
NROWS = KK * 12  # 108 score rows, row m = j*12 + h*2 + half

# G0 padded image geometry: rows y in [-2,58), cols x in [-2,58)
PADH = PADW = 60
ROW0 = COL0 = 2
# G1 dup geometry: [128, 32, 60]; lower p<64: y in [-2,30); upper: y in [26,58)
PADH1 = 32


def _build_consts():
    """Selector constants for the [108, 1568] score layout.

    Score row m = j*12 + h*2 + half  (j in [0,9), h in [0,6), half in {0,1}).
    """
    consts = {}
    NR = 12 * KK  # 108
    # selA0w: [128, 9, 109]; window [:, j, 1:109] = half0, [:, j, 0:108] = half1
    a = np.zeros((128, KK, NR + 1), np.float32)
    for p in range(128):
        for j in range(KK):
            a[p, j, j * 12 + (p // HD) * 2 + 1] = 1.0
    consts["selA0w"] = a.reshape(128, KK * (NR + 1))
    # selA1: [128, 9, 108] for the G1 dup prod (half encoded in partition)
    a = np.zeros((128, KK, NR), np.float32)
    for p in range(128):
        hh = (4 + (p % 64) // HD) * 2 + p // 64
        for j in range(KK):
            a[p, j, j * 12 + hh] = 1.0
    consts["selA1"] = a.reshape(128, KK * NR)
    # selB0lo/hi: [108, 9, 128] lhsT for G0 attn broadcast
    for half in (0, 1):
        b = np.zeros((NR, KK, 128), np.float32)
        for j in range(KK):
            for p in range(128):
                b[j * 12 + (p // HD) * 2 + half, j, p] = 1.0
        consts[f"selB0h{half}"] = b.reshape(NR, KK * 128)
    # selB1: [108, 9, 128] attn broadcast for G1 dup (half from partition)
    b = np.zeros((NR, KK, 128), np.float32)
    for j in range(KK):
        for p in range(128):
            b[j * 12 + (4 + (p % 64) // HD) * 2 + p // 64, j, p] = 1.0
    consts["selB1"] = b.reshape(NR, KK * 128)
    # selD: [108, 12] sum over j per (head, half)
    d = np.zeros((NR, 12), np.float32)
    for m in range(NR):
        d[m, m % 12] = 1.0
    consts["selD"] = d
    # ident: [128, 128] identity for PE pass-through accumulation
    consts["ident"] = np.eye(128, dtype=np.float32)
    # selN: [12, 108] broadcast per-(head,half) value to all j rows
    n = np.zeros((12, NR), np.float32)
    for m in range(NR):
        n[m % 12, m] = 1.0
    consts["selN"] = n
    return consts


# constants kept in plain fp32 (fp32 matmul path) instead of bf16
_F32R_CONSTS = set()


def _bank_chunks(c0, c1):
    """Split [c0,c1) at 512-element PSUM bank boundaries."""
    out = []
    while c0 < c1:
        nxt = min((c0 // 512 + 1) * 512, c1)
        out.append((c0, nxt))
        c0 = nxt
    return out


# stage-C path/engine assignment per (g, j, qi):
#   returns (evac, mult_engine): evac True -> Act copies AB psum -> bf16 sbuf,
#   mult at bf16 2x; evac False -> mult engine reads AB psum at fp32 rate.
def _c_path(i):
    # i = stream position 0..53; alternate paths along the stream so no
    # single engine saturates within a stretch.
    if i % 7 == 3:
        return (True, "gpsimd")  # 8 units on Pool (evac'd; Pool can't read PSUM)
    if i % 3 == 1:
        return (False, "vector")  # ~15 direct on DVE (PSUM operand, fp32 rate)
    return (True, "vector")  # rest evac'd, bf16 2x on DVE


# stage-A mult engine per pass index (0..26)
_A_POOL = {3, 8, 12, 16, 20, 24}


def _a_eng(idx):
    return "gpsimd" if idx in _A_POOL else "vector"


def build_module():
    import concourse.bacc as bacc
    import concourse.mybir as mybir
    import concourse.tile as tile

    fp32 = mybir.dt.float32
    bf16 = mybir.dt.bfloat16
    f32r = mybir.dt.float32r
    AL = mybir.AluOpType
    AF = mybir.ActivationFunctionType

    nc = bacc.Bacc("TRN2", target_bir_lowering=False, debug=False, num_devices=B)

    q_d = nc.dram_tensor("q", [C, H, W], bf16, kind="ExternalInput")
    k_d = nc.dram_tensor("k", [C, H, W], bf16, kind="ExternalInput")
    v_d = nc.dram_tensor("v", [C, H, W], bf16, kind="ExternalInput")
    o_d = nc.dram_tensor("o", [C, HWPIX], bf16, kind="ExternalOutput")
    consts = _build_consts()
    c_d = {
        name: nc.dram_tensor(
            name, list(arr.shape), fp32 if name in _F32R_CONSTS else bf16,
            kind="ExternalInput",
        )
        for name, arr in consts.items()
    }

    with tile.TileContext(nc) as tc:
        with (
            tc.tile_pool(name="io", bufs=2) as io_pool,
            tc.tile_pool(name="work", bufs=2) as work_pool,
            tc.tile_pool(name="tree", bufs=3) as tree_pool,
            tc.tile_pool(name="small", bufs=1) as small_pool,
        ):
            def load_g1_dup(dst_name, src_d, eng, eng2=None):
                """[128, 32, 60] dup tile: lower y in [-2,30), upper y in [26,58)."""
                t = io_pool.tile([128, PADH1, PADW], bf16, tag="kv", name=dst_name)
                nc.gpsimd.memset(t[0:64, 0:ROW0, :], 0.0)
                nc.gpsimd.memset(t[64:128, 30:32, :], 0.0)
                nc.gpsimd.memset(t[:, :, 0:COL0], 0.0)
                nc.gpsimd.memset(t[:, :, COL0 + W :], 0.0)
                eng.dma_start(t[0:64, ROW0 : ROW0 + 30, COL0 : COL0 + W], src_d[128:192, 0:30, :])
                (eng2 or eng).dma_start(t[64:128, 0:30, COL0 : COL0 + W], src_d[128:192, 26:56, :])
                return t

            def load_g1_q(eng, eng2=None):
                t = io_pool.tile([128, HROWS, W], bf16, tag="q", name="q1")
                eng.dma_start(t[0:64, :, :], q_d[128:192, 0:HROWS, :])
                (eng2 or eng).dma_start(t[64:128, :, :], q_d[128:192, HROWS:H, :])
                return t

            def load_g0_pad(dst_name, src_d, eng, eng2=None):
                t = io_pool.tile([128, PADH, PADW], bf16, tag="kv", name=dst_name)
                nc.gpsimd.memset(t[:, 0:ROW0, :], 0.0)
                nc.gpsimd.memset(t[:, ROW0 + H :, :], 0.0)
                nc.gpsimd.memset(t[:, ROW0 : ROW0 + H, 0:COL0], 0.0)
                nc.gpsimd.memset(t[:, ROW0 : ROW0 + H, COL0 + W :], 0.0)
                if eng2 is None:
                    eng.dma_start(t[:, ROW0 : ROW0 + H, COL0 : COL0 + W], src_d[0:128, :, :])
                else:
                    eng.dma_start(
                        t[:, ROW0 : ROW0 + HROWS, COL0 : COL0 + W], src_d[0:128, 0:HROWS, :]
                    )
                    eng2.dma_start(
                        t[:, ROW0 + HROWS : ROW0 + H, COL0 : COL0 + W],
                        src_d[0:128, HROWS:H, :],
                    )
                return t

            sel_sb = {}

            def load_const(name, eng):
                arr = consts[name]
                dt = fp32 if name in _F32R_CONSTS else bf16
                t = small_pool.tile(list(arr.shape), dt, tag=f"c_{name}", name=f"c_{name}")
                eng.dma_start(t[:], c_d[name][:])
                sel_sb[name] = t

            with tc.high_priority():
                load_const("selA1", nc.scalar)
                k1 = load_g1_dup("k1", k_d, nc.sync, nc.scalar)
                q1 = load_g1_q(nc.scalar, nc.sync)

            E_sb0 = small_pool.tile([NROWS, HALF // 2], bf16, tag="E0")
            E_sb1 = small_pool.tile([NROWS, HALF // 2], bf16, tag="E1")
            E_half = (E_sb0, E_sb1)

            # ---- stage A: scores. S_ps[m=(j*12+h*2+half), px] = sum_d q*k_shift
            selA1 = sel_sb["selA1"].rearrange("p (j m) -> p j m", j=KK)
            QW = HALF // 2  # 784
            SROWS = HROWS // 2  # 14

            q0 = io_pool.tile([128, H, W], bf16, tag="q", name="q0")
            nc.scalar.dma_start(q0[:], q_d[0:128, :, :])
            k0 = load_g0_pad("k0", k_d, nc.sync, nc.scalar)
            for name, eng in (
                ("selA0w", nc.scalar), ("selD", nc.sync), ("selN", nc.sync),
                ("selB0h0", nc.scalar), ("selB0h1", nc.sync),
                ("selB1", nc.sync), ("ident", nc.sync),
            ):
                load_const(name, eng)

            passes = [("g1", j) for j in range(KK)]
            for j in range(KK):
                passes += [("g0", j, 0), ("g0", j, 1)]

            def pass_views(p):
                di, dj = SHIFTS[p[1]]
                if p[0] == "g1":
                    qv = q1[:]
                    kv = k1[:, ROW0 + di : ROW0 + di + HROWS, COL0 + dj : COL0 + dj + W]
                else:
                    r0 = p[2] * HROWS
                    qv = q0[:, r0 : r0 + HROWS, :]
                    kv = k0[:, ROW0 + di + r0 : ROW0 + di + r0 + HROWS,
                            COL0 + dj : COL0 + dj + W]
                return qv, kv

            selB0h = [
                sel_sb["selB0h0"].rearrange("m (j p) -> m j p", j=KK),
                sel_sb["selB0h1"].rearrange("m (j p) -> m j p", j=KK),
            ]
            selB1 = sel_sb["selB1"].rearrange("m (j p) -> m j p", j=KK)

            with tc.tile_pool(name="psS", bufs=1, space="PSUM") as psS_pool:
                S_ps = psS_pool.tile([NROWS, HALF], fp32, tag="S")
                prods = {}
                for idx, p in enumerate(passes):
                    prod = work_pool.tile(
                        [128, HROWS, W], bf16, tag="prod", bufs=27, name=f"aprod{idx}"
                    )
                    qv, kv = pass_views(p)
                    getattr(nc, _a_eng(idx)).tensor_tensor(prod[:], qv, kv, AL.mult)
                    prods[idx] = prod

                selA0w = sel_sb["selA0w"].rearrange("p (j m) -> p j m", j=KK)
                selA0h = [selA0w[:, :, 1 : NROWS + 1], selA0w[:, :, 0:NROWS]]
                for idx, p in enumerate(passes):
                    sel = selA1[:, p[1], :] if p[0] == "g1" else selA0h[p[2]][:, p[1], :]
                    pflat = prods[idx].rearrange("p a b -> p (a b)")
                    for n0, n1 in _bank_chunks(0, HALF):
                        nc.tensor.matmul(
                            S_ps[:, n0:n1], sel, pflat[:, n0:n1],
                            start=(idx == 0), stop=(idx == len(passes) - 1),
                        )

                # prefetch v while PE drains the last score matmuls
                v0 = load_g0_pad("v0", v_d, nc.sync, nc.scalar)
                v1 = load_g1_dup("v1", v_d, nc.scalar)

                # exp(scale * S) -> E bf16, evacuating PSUM
                for s in (0, 1):
                    nc.scalar.activation(
                        E_half[s][:], S_ps[:, s * (HALF // 2) : (s + 1) * (HALF // 2)],
                        AF.Exp, scale=float(SCALE),
                    )

            with tc.tile_pool(name="psC", bufs=1, space="PSUM") as ps_pool:
                def emit_B(s):
                    """Normalize E[:, s-chunk]: selD sum, reciprocal, selN
                    broadcast, in-place multiply. Chunked at bank granularity
                    so the two sub-chains pipeline across engines. PSUM via
                    the AB slot ring."""
                    Et = E_half[s]
                    D_ps = ps_pool.tile([12, QW], fp32, tag="AB", bufs=3, name=f"D{s}")
                    RB_ps = ps_pool.tile([NROWS, QW], fp32, tag="AB", bufs=3, name=f"RB{s}")
                    R_ch = small_pool.tile([12, QW], fp32, tag="R", bufs=2)
                    Rb = small_pool.tile([12, QW], bf16, tag="Rb", bufs=2)
                    for c0, c1 in _bank_chunks(0, QW):
                        nc.tensor.matmul(
                            D_ps[:, c0:c1], sel_sb["selD"][:],
                            Et[:, c0:c1], start=True, stop=True,
                        )
                        nc.vector.reciprocal_approx_fast(R_ch[:, c0:c1], D_ps[:, c0:c1])
                        nc.scalar.activation(Rb[:, c0:c1], R_ch[:, c0:c1], AF.Copy)
                        nc.tensor.matmul(
                            RB_ps[:, c0:c1], sel_sb["selN"][:], Rb[:, c0:c1],
                            start=True, stop=True,
                        )
                        nc.vector.tensor_tensor(
                            Et[:, c0:c1], Et[:, c0:c1], RB_ps[:, c0:c1], AL.mult,
                        )

                ident = sel_sb["ident"]
                acc_tiles = {}

                def emit_front(g, j, qi, i):
                    """AB matmuls + evac + mult for unit (g,j,qi); returns prod."""
                    hf = g if g < 2 else 0
                    sel = selB1 if g == 2 else selB0h[hf]
                    vt = v1 if g == 2 else v0
                    di, dj = SHIFTS[j]
                    evac, meng = _c_path(i)
                    ab_ps = ps_pool.tile([128, QW], fp32, tag="AB", bufs=3)
                    for n0, n1 in _bank_chunks(0, QW):
                        nc.tensor.matmul(
                            ab_ps[:, n0:n1], sel[:, j, :],
                            E_half[qi][:, n0:n1], start=True, stop=True,
                        )
                    r0 = ROW0 + di + (hf * HROWS if g < 2 else 0) + qi * SROWS
                    vv = vt[:, r0 : r0 + SROWS, COL0 + dj : COL0 + dj + W]
                    prod = tree_pool.tile(
                        [128, SROWS, W], bf16, tag="prod", bufs=16, name="cprod"
                    )
                    if evac:
                        ab_sb = tree_pool.tile([128, QW], bf16, tag="absb", bufs=16)
                        nc.scalar.activation(ab_sb[:], ab_ps[:], AF.Copy)
                        m_in = ab_sb.rearrange("p (a b) -> p a b", a=SROWS)
                    else:
                        m_in = ab_ps.rearrange("p (a b) -> p a b", a=SROWS)
                    getattr(nc, meng).tensor_tensor(prod[:], m_in, vv, AL.mult)
                    return prod

                def emit_back(g, j, qi, prod):
                    """ACC accumulate for unit; on j==8 evacuate + DMA out."""
                    hf = g if g < 2 else 0
                    key = (g, qi)
                    if j == 0:
                        acc_tiles[key] = [
                            ps_pool.tile(
                                [128, n1 - n0], fp32, tag="ACCb", bufs=2,
                                name=f"ACC{g}{qi}c{n0}",
                            )
                            for n0, n1 in _bank_chunks(0, QW)
                        ]
                    pf = prod.rearrange("p a b -> p (a b)")
                    for ci, (n0, n1) in enumerate(_bank_chunks(0, QW)):
                        nc.tensor.matmul(
                            acc_tiles[key][ci][:], ident[:], pf[:, n0:n1],
                            start=(j == 0), stop=(j == KK - 1),
                        )
                    if j == KK - 1:
                        t_sb = tree_pool.tile([128, QW], bf16, tag="tout", bufs=4)
                        for ci, (n0, n1) in enumerate(_bank_chunks(0, QW)):
                            nc.vector.tensor_copy(
                                t_sb[:, n0:n1], acc_tiles[key][ci][:]
                            )
                        c0 = hf * HALF + qi * QW
                        if g < 2:
                            nc.sync.dma_start(o_d[0:128, c0 : c0 + QW], t_sb[:])
                        else:
                            nc.sync.dma_start(o_d[128:192, c0 : c0 + QW], t_sb[0:64, :])
                            nc.scalar.dma_start(
                                o_d[128:192, HALF + c0 : HALF + c0 + QW], t_sb[64:128, :]
                            )

                # G1 (two output DMAs) first within each stripe so the
                # final block drains through a single output DMA chain.
                units = [(g, j, qi) for qi in (0, 1) for g in (2, 0, 1) for j in range(KK)]
                SK = 14  # pipeline skew: ACC(u) emitted SK units after AB(u)
                emit_B(0)
                queue = []
                for i, u in enumerate(units):
                    if i == 4:
                        emit_B(1)
                    sk = SK if i < 44 else 6
                    while len(queue) >= sk:
                        uu, pp = queue.pop(0)
                        emit_back(*uu, pp)
                    prod = emit_front(*u, i)
                    queue.append((u, prod))
                for uu, pp in queue:
                    emit_back(*uu, pp)

    nc.compile()
    return nc, consts


_CACHE = {}


def _get_module():
    if "nc" not in _CACHE:
        _CACHE["nc"], _CACHE["consts"] = build_module()
    return _CACHE["nc"], _CACHE["consts"]


def make_in_maps(q, k, v, consts):
    import ml_dtypes

    bf = ml_dtypes.bfloat16
    cb = {}
    for name, arr in consts.items():
        cb[name] = arr.astype(np.float32) if name in _F32R_CONSTS else arr.astype(bf)
    qb = np.asarray(q).astype(bf)
    kb = np.asarray(k).astype(bf)
    vb = np.asarray(v).astype(bf)
    in_maps = []
    for b in range(B):
        m = {
            "q": np.ascontiguousarray(qb[b].reshape(C, H, W)),
            "k": np.ascontiguousarray(kb[b].reshape(C, H, W)),
            "v": np.ascontiguousarray(vb[b].reshape(C, H, W)),
        }
        m.update(cb)
        in_maps.append(m)
    return in_maps


def postprocess(raw_o):
    """[C, HWPIX] bf16 per core -> [H, W, C] fp32."""
    return (
        np.asarray(raw_o).astype(np.float32).reshape(C, H, W).transpose(1, 2, 0)
    )


def kernel(q: np.ndarray, k: np.ndarray, v: np.ndarray) -> np.ndarray:
    from concourse import bass_utils

    nc, consts = _get_module()
    in_maps = make_in_maps(np.asarray(q), np.asarray(k), np.asarray(v), consts)
    res = bass_utils.run_bass_kernel_spmd(nc, in_maps, core_ids=list(range(B)))
    out = np.stack([postprocess(r["o"]) for r in res.results])
    return out


# revision 36
# speedup vs baseline: 1.0340x; 1.0163x over previous
"""DilateAttention Trainium2 kernel (nn_DilateAttention), v2 bf16.

Full inputs q,k,v: [8, 192, 56, 56] fp32. Output: [8, 56, 56, 192] fp32.
Sharded data-parallel over batch B=8 across 8 NeuronCores.

Per-core layout: channels-on-partitions, bf16 datapath. Head group G0
(heads 0-3) fills 128 partitions; group G1 (heads 4-5, 64 ch) is pixel-split
across partitions (image halves stacked).

Pipeline per core:
  A: prod_j = q*k_shift (DVE/Pool bf16 2x)  ->  PE selector-reduce -> S PSUM
  exp on Act -> E bf16 SBUF;  B: selD-matmul sums, reciprocal, selN-matmul
  broadcast, normalize (E' bf16)
  C: per (group, j): PE broadcast E'->AB PSUM; Act evacuates AB to bf16 SBUF
  (or DVE reads PSUM directly); DVE/Pool multiply with v; PE identity-matmul
  accumulates over j into ACC PSUM.
  Output: Act converts ACC->bf16, DMA channel-major; host transposes to
  [B,H,W,C] fp32.
"""

import sys

for _p in ("/opt/trn_rl_repo",):
    if _p not in sys.path:
        sys.path.insert(0, _p)

import numpy as np

B = 8
C = 192
H = W = 56
HD = 32
NH = 6  # heads
KK = 9  # kernel*kernel
SCALE = HD ** -0.5
HWPIX = H * W  # 3136
HALF = HWPIX // 2  # 1568
HROWS = H // 2  # 28
SHIFTS = [(di, dj) for di in (-2, 0, 2) for dj in (-2, 0, 2)]
NROWS = KK * 12  # 108 score rows, row m = j*12 + h*2 + half

# G0 padded image geometry: rows y in [-2,58), cols x in [-2,58)
PADH = PADW = 60
ROW0 = COL0 = 2
# G1 dup geometry: [128, 32, 60]; lower p<64: y in [-2,30); upper: y in [26,58)
PADH1 = 32


def _build_consts():
    """Selector constants for the [108, 1568] score layout.

    Score row m = j*12 + h*2 + half  (j in [0,9), h in [0,6), half in {0,1}).
    """
    consts = {}
    NR = 12 * KK  # 108
    # selA0w: [128, 9, 109]; window [:, j, 1:109] = half0, [:, j, 0:108] = half1
    a = np.zeros((128, KK, NR + 1), np.float32)
    for p in range(128):
        for j in range(KK):
            a[p, j, j * 12 + (p // HD) * 2 + 1] = 1.0
    consts["selA0w"] = a.reshape(128, KK * (NR + 1))
    # selA1: [128, 9, 108] for the G1 dup prod (half encoded in partition)
    a = np.zeros((128, KK, NR), np.float32)
    for p in range(128):
        hh = (4 + (p % 64) // HD) * 2 + p // 64
        for j in range(KK):
            a[p, j, j * 12 + hh] = 1.0
    consts["selA1"] = a.reshape(128, KK * NR)
    # selB0lo/hi: [108, 9, 128] lhsT for G0 attn broadcast
    for half in (0, 1):
        b = np.zeros((NR, KK, 128), np.float32)
        for j in range(KK):
            for p in range(128):
                b[j * 12 + (p // HD) * 2 + half, j, p] = 1.0
        consts[f"selB0h{half}"] = b.reshape(NR, KK * 128)
    # selB1: [108, 9, 128] attn broadcast for G1 dup (half from partition)
    b = np.zeros((NR, KK, 128), np.float32)
    for j in range(KK):
        for p in range(128):
            b[j * 12 + (4 + (p % 64) // HD) * 2 + p // 64, j, p] = 1.0
    consts["selB1"] = b.reshape(NR, KK * 128)
    # selD: [108, 12] sum over j per (head, half)
    d = np.zeros((NR, 12), np.float32)
    for m in range(NR):
        d[m, m % 12] = 1.0
    consts["selD"] = d
    # ident: [128, 128] identity for PE pass-through accumulation
    consts["ident"] = np.eye(128, dtype=np.float32)
    # selN: [12, 108] broadcast per-(head,half) value to all j rows
    n = np.zeros((12, NR), np.float32)
    for m in range(NR):
        n[m % 12, m] = 1.0
    consts["selN"] = n
    return consts


# constants kept in plain fp32 (fp32 matmul path) instead of bf16
_F32R_CONSTS = set()


def _bank_chunks(c0, c1):
    """Split [c0,c1) at 512-element PSUM bank boundaries."""
    out = []
    while c0 < c1:
        nxt = min((c0 // 512 + 1) * 512, c1)
        out.append((c0, nxt))
        c0 = nxt
    return out


# stage-C path/engine assignment per (g, j, qi):
#   returns (evac, mult_engine): evac True -> Act copies AB psum -> bf16 sbuf,
#   mult at bf16 2x; evac False -> mult engine reads AB psum at fp32 rate.
def _c_path(i):
    # i = stream position 0..53; alternate paths along the stream so no
    # single engine saturates within a stretch.
    if i % 7 == 3:
        return (True, "gpsimd")  # 8 units on Pool (evac'd; Pool can't read PSUM)
    if i % 5 == 1:
        return (False, "vector")  # ~10 direct on DVE (PSUM operand, fp32 rate)
    return (True, "vector")  # rest evac'd, bf16 2x on DVE


# stage-A mult engine per pass index (0..26)
_A_POOL = {3, 8, 12, 16, 20, 24}


def _a_eng(idx):
    return "gpsimd" if idx in _A_POOL else "vector"


def build_module():
    import concourse.bacc as bacc
    import concourse.mybir as mybir
    import concourse.tile as tile

    fp32 = mybir.dt.float32
    bf16 = mybir.dt.bfloat16
    f32r = mybir.dt.float32r
    AL = mybir.AluOpType
    AF = mybir.ActivationFunctionType

    nc = bacc.Bacc("TRN2", target_bir_lowering=False, debug=False, num_devices=B)

    q_d = nc.dram_tensor("q", [C, H, W], bf16, kind="ExternalInput")
    k_d = nc.dram_tensor("k", [C, H, W], bf16, kind="ExternalInput")
    v_d = nc.dram_tensor("v", [C, H, W], bf16, kind="ExternalInput")
    o_d = nc.dram_tensor("o", [C, HWPIX], bf16, kind="ExternalOutput")
    consts = _build_consts()
    c_d = {
        name: nc.dram_tensor(
            name, list(arr.shape), fp32 if name in _F32R_CONSTS else bf16,
            kind="ExternalInput",
        )
        for name, arr in consts.items()
    }

    with tile.TileContext(nc) as tc:
        with (
            tc.tile_pool(name="io", bufs=2) as io_pool,
            tc.tile_pool(name="work", bufs=2) as work_pool,
            tc.tile_pool(name="tree", bufs=3) as tree_pool,
            tc.tile_pool(name="small", bufs=1) as small_pool,
        ):
            def load_g1_dup(dst_name, src_d, eng, eng2=None):
                """[128, 32, 60] dup tile: lower y in [-2,30), upper y in [26,58)."""
                t = io_pool.tile([128, PADH1, PADW], bf16, tag="kv", name=dst_name)
                nc.gpsimd.memset(t[0:64, 0:ROW0, :], 0.0)
                nc.gpsimd.memset(t[64:128, 30:32, :], 0.0)
                nc.gpsimd.memset(t[:, :, 0:COL0], 0.0)
                nc.gpsimd.memset(t[:, :, COL0 + W :], 0.0)
                eng.dma_start(t[0:64, ROW0 : ROW0 + 30, COL0 : COL0 + W], src_d[128:192, 0:30, :])
                (eng2 or eng).dma_start(t[64:128, 0:30, COL0 : COL0 + W], src_d[128:192, 26:56, :])
                return t

            def load_g1_q(eng, eng2=None):
                t = io_pool.tile([128, HROWS, W], bf16, tag="q", name="q1")
                eng.dma_start(t[0:64, :, :], q_d[128:192, 0:HROWS, :])
                (eng2 or eng).dma_start(t[64:128, :, :], q_d[128:192, HROWS:H, :])
                return t

            def load_g0_pad(dst_name, src_d, eng, eng2=None):
                t = io_pool.tile([128, PADH, PADW], bf16, tag="kv", name=dst_name)
                nc.gpsimd.memset(t[:, 0:ROW0, :], 0.0)
                nc.gpsimd.memset(t[:, ROW0 + H :, :], 0.0)
                nc.gpsimd.memset(t[:, ROW0 : ROW0 + H, 0:COL0], 0.0)
                nc.gpsimd.memset(t[:, ROW0 : ROW0 + H, COL0 + W :], 0.0)
                if eng2 is None:
                    eng.dma_start(t[:, ROW0 : ROW0 + H, COL0 : COL0 + W], src_d[0:128, :, :])
                else:
                    eng.dma_start(
                        t[:, ROW0 : ROW0 + HROWS, COL0 : COL0 + W], src_d[0:128, 0:HROWS, :]
                    )
                    eng2.dma_start(
                        t[:, ROW0 + HROWS : ROW0 + H, COL0 : COL0 + W],
                        src_d[0:128, HROWS:H, :],
                    )
                return t

            sel_sb = {}

            def load_const(name, eng):
                arr = consts[name]
                dt = fp32 if name in _F32R_CONSTS else bf16
                t = small_pool.tile(list(arr.shape), dt, tag=f"c_{name}", name=f"c_{name}")
                eng.dma_start(t[:], c_d[name][:])
                sel_sb[name] = t

            with tc.high_priority():
                load_const("selA1", nc.scalar)
                k1 = load_g1_dup("k1", k_d, nc.sync, nc.scalar)
                q1 = load_g1_q(nc.scalar, nc.sync)

            E_sb0 = small_pool.tile([NROWS, HALF // 2], bf16, tag="E0")
            E_sb1 = small_pool.tile([NROWS, HALF // 2], bf16, tag="E1")
            E_half = (E_sb0, E_sb1)

            # ---- stage A: scores. S_ps[m=(j*12+h*2+half), px] = sum_d q*k_shift
            selA1 = sel_sb["selA1"].rearrange("p (j m) -> p j m", j=KK)
            QW = HALF // 2  # 784
            SROWS = HROWS // 2  # 14

            q0 = io_pool.tile([128, H, W], bf16, tag="q", name="q0")
            nc.scalar.dma_start(q0[:], q_d[0:128, :, :])
            k0 = load_g0_pad("k0", k_d, nc.sync, nc.scalar)
            for name, eng in (
                ("selA0w", nc.scalar), ("selD", nc.sync), ("selN", nc.sync),
                ("selB0h0", nc.scalar), ("selB0h1", nc.sync),
                ("selB1", nc.sync), ("ident", nc.sync),
            ):
                load_const(name, eng)

            passes = [("g1", j) for j in range(KK)]
            for j in range(KK):
                passes += [("g0", j, 0), ("g0", j, 1)]

            def pass_views(p):
                di, dj = SHIFTS[p[1]]
                if p[0] == "g1":
                    qv = q1[:]
                    kv = k1[:, ROW0 + di : ROW0 + di + HROWS, COL0 + dj : COL0 + dj + W]
                else:
                    r0 = p[2] * HROWS
                    qv = q0[:, r0 : r0 + HROWS, :]
                    kv = k0[:, ROW0 + di + r0 : ROW0 + di + r0 + HROWS,
                            COL0 + dj : COL0 + dj + W]
                return qv, kv

            selB0h = [
                sel_sb["selB0h0"].rearrange("m (j p) -> m j p", j=KK),
                sel_sb["selB0h1"].rearrange("m (j p) -> m j p", j=KK),
            ]
            selB1 = sel_sb["selB1"].rearrange("m (j p) -> m j p", j=KK)

            with tc.tile_pool(name="psS", bufs=1, space="PSUM") as psS_pool:
                S_ps = psS_pool.tile([NROWS, HALF], fp32, tag="S")
                prods = {}
                for idx, p in enumerate(passes):
                    prod = work_pool.tile(
                        [128, HROWS, W], bf16, tag="prod", bufs=27, name=f"aprod{idx}"
                    )
                    qv, kv = pass_views(p)
                    getattr(nc, _a_eng(idx)).tensor_tensor(prod[:], qv, kv, AL.mult)
                    prods[idx] = prod

                selA0w = sel_sb["selA0w"].rearrange("p (j m) -> p j m", j=KK)
                selA0h = [selA0w[:, :, 1 : NROWS + 1], selA0w[:, :, 0:NROWS]]
                for idx, p in enumerate(passes):
                    sel = selA1[:, p[1], :] if p[0] == "g1" else selA0h[p[2]][:, p[1], :]
                    pflat = prods[idx].rearrange("p a b -> p (a b)")
                    for n0, n1 in _bank_chunks(0, HALF):
                        nc.tensor.matmul(
                            S_ps[:, n0:n1], sel, pflat[:, n0:n1],
                            start=(idx == 0), stop=(idx == len(passes) - 1),
                        )

                # prefetch v while PE drains the last score matmuls
                v0 = load_g0_pad("v0", v_d, nc.sync, nc.scalar)
                v1 = load_g1_dup("v1", v_d, nc.scalar)

                # exp(scale * S) -> E bf16, evacuating PSUM
                for s in (0, 1):
                    nc.scalar.activation(
                        E_half[s][:], S_ps[:, s * (HALF // 2) : (s + 1) * (HALF // 2)],
                        AF.Exp, scale=float(SCALE),
                    )

            with tc.tile_pool(name="psC", bufs=1, space="PSUM") as ps_pool:
                def emit_B(s):
                    """Normalize E[:, s-chunk]: selD sum, reciprocal, selN
                    broadcast, in-place multiply. Chunked at bank granularity
                    so the two sub-chains pipeline across engines. PSUM via
                    the AB slot ring."""
                    Et = E_half[s]
                    D_ps = ps_pool.tile([12, QW], fp32, tag="AB", bufs=3, name=f"D{s}")
                    RB_ps = ps_pool.tile([NROWS, QW], fp32, tag="AB", bufs=3, name=f"RB{s}")
                    R_ch = small_pool.tile([12, QW], fp32, tag="R", bufs=2)
                    Rb = small_pool.tile([12, QW], bf16, tag="Rb", bufs=2)
                    for c0, c1 in _bank_chunks(0, QW):
                        nc.tensor.matmul(
                            D_ps[:, c0:c1], sel_sb["selD"][:],
                            Et[:, c0:c1], start=True, stop=True,
                        )
                        nc.vector.reciprocal_approx_fast(R_ch[:, c0:c1], D_ps[:, c0:c1])
                        nc.scalar.activation(Rb[:, c0:c1], R_ch[:, c0:c1], AF.Copy)
                        nc.tensor.matmul(
                            RB_ps[:, c0:c1], sel_sb["selN"][:], Rb[:, c0:c1],
                            start=True, stop=True,
                        )
                        nc.vector.tensor_tensor(
                            Et[:, c0:c1], Et[:, c0:c1], RB_ps[:, c0:c1], AL.mult,
                        )

                ident = sel_sb["ident"]
                acc_tiles = {}

                def emit_front(g, j, qi, i):
                    """AB matmuls + evac + mult for unit (g,j,qi); returns prod."""
                    hf = g if g < 2 else 0
                    sel = selB1 if g == 2 else selB0h[hf]
                    vt = v1 if g == 2 else v0
                    di, dj = SHIFTS[j]
                    evac, meng = _c_path(i)
                    ab_ps = ps_pool.tile([128, QW], fp32, tag="AB", bufs=3)
                    for n0, n1 in _bank_chunks(0, QW):
                        nc.tensor.matmul(
                            ab_ps[:, n0:n1], sel[:, j, :],
                            E_half[qi][:, n0:n1], start=True, stop=True,
                        )
                    r0 = ROW0 + di + (hf * HROWS if g < 2 else 0) + qi * SROWS
                    vv = vt[:, r0 : r0 + SROWS, COL0 + dj : COL0 + dj + W]
                    prod = tree_pool.tile(
                        [128, SROWS, W], bf16, tag="prod", bufs=16, name="cprod"
                    )
                    if evac:
                        ab_sb = tree_pool.tile([128, QW], bf16, tag="absb", bufs=16)
                        nc.scalar.activation(ab_sb[:], ab_ps[:], AF.Copy)
                        m_in = ab_sb.rearrange("p (a b) -> p a b", a=SROWS)
                    else:
                        m_in = ab_ps.rearrange("p (a b) -> p a b", a=SROWS)
                    getattr(nc, meng).tensor_tensor(prod[:], m_in, vv, AL.mult)
                    return prod

                def emit_back(g, j, qi, prod):
                    """ACC accumulate for unit; on j==8 evacuate + DMA out."""
                    hf = g if g < 2 else 0
                    key = (g, qi)
                    if j == 0:
                        acc_tiles[key] = [
                            ps_pool.tile(
                                [128, n1 - n0], fp32, tag="ACCb", bufs=2,
                                name=f"ACC{g}{qi}c{n0}",
                            )
                            for n0, n1 in _bank_chunks(0, QW)
                        ]
                    pf = prod.rearrange("p a b -> p (a b)")
                    for ci, (n0, n1) in enumerate(_bank_chunks(0, QW)):
                        nc.tensor.matmul(
                            acc_tiles[key][ci][:], ident[:], pf[:, n0:n1],
                            start=(j == 0), stop=(j == KK - 1),
                        )
                    if j == KK - 1:
                        t_sb = tree_pool.tile([128, QW], bf16, tag="tout", bufs=4)
                        for ci, (n0, n1) in enumerate(_bank_chunks(0, QW)):
                            nc.vector.tensor_copy(
                                t_sb[:, n0:n1], acc_tiles[key][ci][:]
                            )
                        c0 = hf * HALF + qi * QW
                        if g < 2:
                            nc.sync.dma_start(o_d[0:128, c0 : c0 + QW], t_sb[:])
                        else:
                            nc.sync.dma_start(o_d[128:192, c0 : c0 + QW], t_sb[0:64, :])
                            nc.scalar.dma_start(
                                o_d[128:192, HALF + c0 : HALF + c0 + QW], t_sb[64:128, :]
                            )

                # G1 (two output DMAs) first within each stripe so the
                # final block drains through a single output DMA chain.
                units = [(g, j, qi) for qi in (0, 1) for g in (2, 0, 1) for j in range(KK)]
                SK = 14  # pipeline skew: ACC(u) emitted SK units after AB(u)
                emit_B(0)
                queue = []
                for i, u in enumerate(units):
                    if i == 4:
                        emit_B(1)
                    sk = SK if i < 44 else 6
                    while len(queue) >= sk:
                        uu, pp = queue.pop(0)
                        emit_back(*uu, pp)
                    prod = emit_front(*u, i)
                    queue.append((u, prod))
                for uu, pp in queue:
                    emit_back(*uu, pp)

    nc.compile()
    return nc, consts


_CACHE = {}


def _get_module():
    if "nc" not in _CACHE:
        _CACHE["nc"], _CACHE["consts"] = build_module()
    return _CACHE["nc"], _CACHE["consts"]


def make_in_maps(q, k, v, consts):
    import ml_dtypes

    bf = ml_dtypes.bfloat16
    cb = {}
    for name, arr in consts.items():
        cb[name] = arr.astype(np.float32) if name in _F32R_CONSTS else arr.astype(bf)
    qb = np.asarray(q).astype(bf)
    kb = np.asarray(k).astype(bf)
    vb = np.asarray(v).astype(bf)
    in_maps = []
    for b in range(B):
        m = {
            "q": np.ascontiguousarray(qb[b].reshape(C, H, W)),
            "k": np.ascontiguousarray(kb[b].reshape(C, H, W)),
            "v": np.ascontiguousarray(vb[b].reshape(C, H, W)),
        }
        m.update(cb)
        in_maps.append(m)
    return in_maps


def postprocess(raw_o):
    """[C, HWPIX] bf16 per core -> [H, W, C] fp32."""
    return (
        np.asarray(raw_o).astype(np.float32).reshape(C, H, W).transpose(1, 2, 0)
    )


def kernel(q: np.ndarray, k: np.ndarray, v: np.ndarray) -> np.ndarray:
    from concourse import bass_utils

    nc, consts = _get_module()
    in_maps = make_in_maps(np.asarray(q), np.asarray(k), np.asarray(v), consts)
    res = bass_utils.run_bass_kernel_spmd(nc, in_maps, core_ids=list(range(B)))
    out = np.stack([postprocess(r["o"]) for r in res.results])
    return out
